# revision 15
# baseline (speedup 1.0000x reference)
"""Trainium2 Bass kernel for the segment_reduce conv-pyramid problem.

Math: the reference applies 4 levels of a shared Conv2d(3->1, 3x3, SAME) over
sliding windows of the slice axis (13 -> 10 -> 7 -> 4 -> 1), then projects
W (512) down to K=10 and applies sigmoid.  Only x[:, 0:9] influences the
output.  The 4-level pyramid composes into

    gfv[b,h,w] = sum_{s=0..8} sum_{d=0..8} (M[s,d].T @ x[b,s])[h, w+d-4]

where M[s,d] are 64x64 matrices (exact in H, including H borders, computed
by composing the per-level banded matrices).  This is exact for interior
columns w in [4, 507]; the 8 border columns are computed exactly with a
direct 4-level recursion on width-16 strips (batched over all samples).

Performance structure (v2):
  - 2 samples share each PSUM bank (partitions 0:64 / 64:128); their
    stage-A matmuls are issued as concurrent column-tiled pairs
    (tile_position (0,0)/(0,64) for the 128-contraction groups,
    (0,0)/(64,64) for the single-slice group) -> ~2x PE throughput.
  - weight-block-outer loop (45 blocks x 8 in-flight samples) maximizes
    stationary-weight locality.
  - strips framed by memset (no zero-DMA); borders patched into the
    pair-stacked gfv tile; projection transposes run as concurrent
    row-tiled pairs (T0/T8); output DMA'd per-pair straight from the
    sigmoid tile.

Sharding: pure data parallel over the sample axis: 16 samples per core.
"""

import sys
import time

sys.path.insert(0, "/opt/trn_rl_repo")

import numpy as np  # noqa: E402
import ml_dtypes  # noqa: E402
from contextlib import ExitStack  # noqa: E402

import concourse.bass as bass  # noqa: E402
import concourse.bacc as bacc  # noqa: E402
import concourse.tile as tile  # noqa: E402
from concourse import mybir  # noqa: E402
from concourse.bass_utils import run_bass_kernel_spmd  # noqa: E402

NCORES = 8
NB, H, W, K = 128, 64, 512, 10
NS_USED = 9          # slices 9..12 never reach the output
BPC = NB // NCORES   # 16 samples per core
NPAIR = BPC // 2     # 8 sample-pairs per core

F32 = mybir.dt.float32
BF16 = mybir.dt.bfloat16
SIG = mybir.ActivationFunctionType.Sigmoid

TRACE = False
TRACE_DIR = None
LAST_EXEC_NS = None
_PROG_CACHE = {}

D_ORDER = (4, 0, 1, 2, 3, 5, 6, 7, 8)  # zero-shift first: full-width PSUM init


# ----------------------------------------------------------------------------
# host-side weight preparation (tiny, O(levels * 81 * 64^3))
# ----------------------------------------------------------------------------
def _banded(Kw, c, kx):
    """B[c,kx]: 64x64 lhsT-oriented [h_in, h_out] banded matrix."""
    B = np.zeros((H, H))
    for ho in range(H):
        for hi in range(max(0, ho - 1), min(H, ho + 2)):
            B[hi, ho] = Kw[c, hi - ho + 1, kx]
    return B


def _prep_weights(conv_w, proj_w, proj_b):
    Kw = np.asarray(conv_w, dtype=np.float64)[0]  # (3,3,3) [c, ky, kx]
    B = np.zeros((3, 3, H, H))
    for c in range(3):
        for kx in range(3):
            B[c, kx] = _banded(Kw, c, kx)

    # compose 4 levels: state (slice offset s, total x-shift d) -> matrix
    cur = {(0, 0): np.eye(H)}
    for _ in range(4):
        nxt = {}
        for (s, d), Mat in cur.items():
            for c in range(3):
                for kx in range(3):
                    key = (s + c, d + kx)
                    v = Mat @ B[c, kx]
                    if key in nxt:
                        nxt[key] = nxt[key] + v
                    else:
                        nxt[key] = v
        cur = nxt
    M = np.zeros((9, 9, H, H))
    for (s, d), Mat in cur.items():
        M[s, d] = Mat

    # stage-A weights for slice-pair groups g<4: wa[(g*9+d)] = [M[2g,d];
    # M[2g+1,d]] stacked rows, packed into SBUF layout [128, 36*64]
    wa = np.zeros((4, 9, 128, H), np.float32)
    for g in range(4):
        for d in range(9):
            wa[g, d, :H] = M[2 * g, d]
            wa[g, d, H:] = M[2 * g + 1, d]
    wa_sb = np.ascontiguousarray(
        wa.reshape(36, 128, H).transpose(1, 0, 2).reshape(128, 36 * H)
    ).astype(ml_dtypes.bfloat16)

    # stage-A weights for the single-slice group g=4, duplicated in both
    # partition halves so tile T10 (rows 64:128) can read them: [128, 9*64]
    wa4 = np.zeros((9, 128, H), np.float32)
    for d in range(9):
        wa4[d, 0:H] = M[8, d]
        wa4[d, H:] = M[8, d]
    wa4_sb = np.ascontiguousarray(
        wa4.transpose(1, 0, 2).reshape(128, 9 * H)
    ).astype(ml_dtypes.bfloat16)

    # strip level weights (pair-structured), SBUF layout [128, 6*128]
    wl = np.zeros((2, 3, 128, 128), np.float32)
    for kx in range(3):
        wl[0, kx, 0:H, 0:H] = B[0, kx]
        wl[0, kx, H:, 0:H] = B[1, kx]
        wl[0, kx, H:, H:] = B[0, kx]
        wl[1, kx, 0:H, 0:H] = B[2, kx]
        wl[1, kx, 0:H, H:] = B[1, kx]
        wl[1, kx, H:, H:] = B[2, kx]
    wl_sb = np.ascontiguousarray(
        wl.reshape(6, 128, 128).transpose(1, 0, 2).reshape(128, 6 * 128)
    ).astype(ml_dtypes.bfloat16)

    # level-4 weights (single output slice): [ [B0;B1], [B2;0] ] -> [128, 6*64]
    wl4 = np.zeros((2, 3, 128, H), np.float32)
    for kx in range(3):
        wl4[0, kx, 0:H] = B[0, kx]
        wl4[0, kx, H:] = B[1, kx]
        wl4[1, kx, 0:H] = B[2, kx]
    wl4_sb = np.ascontiguousarray(
        wl4.reshape(6, 128, H).transpose(1, 0, 2).reshape(128, 6 * H)
    ).astype(ml_dtypes.bfloat16)

    # projection weights, transposed per 128-chunk: [128, 4*K] bf16
    pw = np.asarray(proj_w, np.float64)  # [K, 512]
    pwt = np.zeros((128, 4 * K), np.float64)
    for c4 in range(4):
        pwt[:, c4 * K : (c4 + 1) * K] = pw[:, c4 * 128 : (c4 + 1) * 128].T
    pwt_sb = pwt.astype(ml_dtypes.bfloat16)

    pb = np.asarray(proj_b, np.float32).reshape(K, 1)

    # identity duplicated in both partition halves for row-tiled transposes
    id2 = np.zeros((128, H), np.float32)
    id2[0:H] = np.eye(H)
    id2[H:] = np.eye(H)
    id2_sb = id2.astype(ml_dtypes.bfloat16)
    idf = np.eye(K, dtype=np.float32)
    return wa_sb, wa4_sb, wl_sb, wl4_sb, pwt_sb, pb, id2_sb, idf


# ----------------------------------------------------------------------------
# device program (SPMD, identical on all 8 cores)
# ----------------------------------------------------------------------------
def _build_program():
    if "nc" in _PROG_CACHE:
        return _PROG_CACHE["nc"]

    nc = bacc.Bacc("TRN2", target_bir_lowering=False, debug=False)
    x9 = nc.dram_tensor("x9", [BPC, NS_USED, H, W], BF16, kind="ExternalInput").ap()
    wa_d = nc.dram_tensor("wa", [128, 36 * H], BF16, kind="ExternalInput").ap()
    wa4_d = nc.dram_tensor("wa4", [128, 9 * H], BF16, kind="ExternalInput").ap()
    wl_d = nc.dram_tensor("wl", [128, 6 * 128], BF16, kind="ExternalInput").ap()
    wl4_d = nc.dram_tensor("wl4", [128, 6 * H], BF16, kind="ExternalInput").ap()
    pwt_d = nc.dram_tensor("pwt", [128, 4 * K], BF16, kind="ExternalInput").ap()
    pb_d = nc.dram_tensor("pb", [K, 1], F32, kind="ExternalInput").ap()
    id2_d = nc.dram_tensor("id2", [128, H], BF16, kind="ExternalInput").ap()
    idf_d = nc.dram_tensor("idf", [K, K], F32, kind="ExternalInput").ap()
    out_d = nc.dram_tensor("out", [BPC, H, K], F32, kind="ExternalOutput").ap()

    with tile.TileContext(nc) as tc, ExitStack() as ctx:
        wp = ctx.enter_context(tc.tile_pool(name="wp", bufs=1))
        sp = ctx.enter_context(tc.tile_pool(name="sp", bufs=1))
        xp = ctx.enter_context(tc.tile_pool(name="xp", bufs=1))
        x8p = ctx.enter_context(tc.tile_pool(name="x8p", bufs=1))
        x8ep = ctx.enter_context(tc.tile_pool(name="x8ep", bufs=1))

        wa_sb = wp.tile([128, 36 * H], BF16)
        wa4_sb = wp.tile([128, 9 * H], BF16)
        wl_sb = wp.tile([128, 6 * 128], BF16)
        wl4_sb = wp.tile([128, 6 * H], BF16)
        pwt_sb = wp.tile([128, 4 * K], BF16)
        pb_sb = wp.tile([K, 1], F32)
        id2_sb = wp.tile([128, H], BF16)
        idf_sb = wp.tile([K, K], F32)
        for dst, src in (
            (wa_sb, wa_d), (wa4_sb, wa4_d), (wl_sb, wl_d), (wl4_sb, wl4_d),
            (pwt_sb, pwt_d), (pb_sb, pb_d), (id2_sb, id2_d), (idf_sb, idf_d),
        ):
            nc.sync.dma_start(out=dst[:], in_=src)

        # persistent strip tiles: [128, ntiles, 32 blocks, 18] where the 18-dim
        # is 16 data cols framed by one zero col each side (exact zero-pad for
        # the W-direction conv shifts).  block = pair*4 + parity*2 + side.
        sx = sp.tile([128, 5, 32, 18], BF16)
        sl1 = sp.tile([128, 4, 32, 18], BF16)
        sl2 = sp.tile([128, 3, 32, 18], BF16)
        sl3 = sp.tile([128, 2, 32, 18], BF16)
        sl4 = sp.tile([H, 32, 16], BF16)
        # pair-stacked gfv: partitions 0:64 even sample, 64:128 odd sample
        gfv_all = sp.tile([128, NPAIR, W], BF16)
        for t in (sx, sl1, sl2, sl3):
            nc.vector.memset(t[:], 0.0)

        # ------------------- loads (g-major per half) + strip extraction ----
        xm_t, x8_t, x8e_t = [], [], []
        for p in range(NPAIR):
            xm = xp.tile([128, 2, 4, W], BF16, tag=f"xm{p}")
            x8st = x8p.tile([128, W], BF16, tag=f"x8{p}")
            x8e = x8ep.tile([H, 2, W], BF16, tag=f"x8e{p}")
            xm_t.append(xm)
            x8_t.append(x8st)
            x8e_t.append(x8e)

        # issue order matches stage-A consumption: half 0's slice-pair groups
        # in g order, then its last slices, then half 1
        for h in range(2):
            for g in range(4):
                for p4 in range(4):
                    p = 4 * h + p4
                    for par in range(2):
                        nc.gpsimd.dma_start(
                            out=xm_t[p][:, par, g, :],
                            in_=x9[2 * p + par, 2 * g : 2 * g + 2].rearrange(
                                "s h w -> (s h) w"
                            ),
                        )
            for p4 in range(4):
                p = 4 * h + p4
                for par in range(2):
                    nc.gpsimd.dma_start(
                        out=x8_t[p][par * H : (par + 1) * H, :],
                        in_=x9[2 * p + par, 8],
                    )
                nc.gpsimd.dma_start(
                    out=x8e_t[p][:],
                    in_=x9[2 * p : 2 * p + 2, 8].rearrange("b h w -> h b w"),
                )

        for p in range(NPAIR):
            for par in range(2):
                for side in range(2):
                    blk = p * 4 + par * 2 + side
                    woff = 0 if side == 0 else W - 16
                    nc.vector.tensor_copy(
                        out=sx[:, 0:4, blk, 1:17],
                        in_=xm_t[p][:, par, 0:4, woff : woff + 16],
                    )
                    nc.vector.tensor_copy(
                        out=sx[0:H, 4, blk, 1:17],
                        in_=x8e_t[p][0:H, par, woff : woff + 16],
                    )

        # ------------------- stage A + strips + projection ------------------
        def stage_a_half(h, gap):
            """45 weight blocks x 8 samples; 2 samples per PSUM bank with
            concurrent column-tiled matmul pairs."""
            banks = []
            for p4 in range(4):
                gbank = gap.tile([128, W], F32, tag=f"g{p4}", name=f"gb{h}_{p4}")
                banks.append(gbank)
            for g in range(5):
                for d in D_ORDER:
                    o = d - 4
                    ol, oh = max(0, -o), W - max(0, o)
                    for p4 in range(4):
                        hp = 4 * h + p4
                        for par in range(2):
                            po = par * H
                            if g < 4:
                                lhsT = wa_sb[:, (g * 9 + d) * H : (g * 9 + d + 1) * H]
                                rhs = xm_t[hp][:, par, g, ol + o : oh + o]
                                tpos = (0, po)
                            else:
                                lhsT = wa4_sb[po : po + H, d * H : (d + 1) * H]
                                rhs = x8_t[hp][po : po + H, ol + o : oh + o]
                                tpos = (po, po)
                            nc.tensor.matmul(
                                banks[p4][po : po + H, ol:oh],
                                lhsT,
                                rhs,
                                start=(g == 0 and d == 4),
                                stop=(g == 4 and d == 8),
                                tile_position=tpos,
                            )
            for p4 in range(4):
                hp = 4 * h + p4
                for par in range(2):
                    po = par * H
                    nc.vector.tensor_copy(
                        out=gfv_all[po : po + H, hp, :],
                        in_=banks[p4][po : po + H, :],
                    )

        def strip_level(dst, src, src_ntile, n_out):
            """One pyramid level on the strips. dst gets ceil(n_out/2) tiles."""
            with tc.tile_pool(name=f"spp{n_out}", bufs=4, space="PSUM") as spp:
                for t in range((n_out + 1) // 2):
                    single = (2 * t + 1) >= n_out
                    cols = H if single else 128
                    ps = spp.tile([128, 32, 16], F32)
                    for widx in (0, 1):  # W1 @ src[t], W2 @ src[t+1]
                        it = t + widx
                        if it >= src_ntile:
                            continue
                        for kx in range(3):
                            o = kx - 1
                            rhs = src[:, it, :, 1 + o : 17 + o]
                            for ch in range(cols // H):
                                co = ch * H
                                lhsT = wl_sb[:, (widx * 3 + kx) * 128 + co :][:, 0:H]
                                nc.tensor.matmul(
                                    ps[co : co + H, :, :],
                                    lhsT,
                                    rhs,
                                    start=(widx == 0 and kx == 0),
                                    stop=(widx == 1 and kx == 2),
                                    tile_position=(0, co),
                                )
                    nc.vector.tensor_copy(
                        out=dst[0:cols, t, :, 1:17], in_=ps[0:cols, :, :]
                    )

        with tc.tile_pool(name="gap", bufs=1, space="PSUM") as gap:
            stage_a_half(0, gap)

            # exact border strips (levels 1..4), PSUM banks 4..7
            strip_level(sl1, sx, 5, 7)
            strip_level(sl2, sl1, 4, 5)
            strip_level(sl3, sl2, 3, 3)
            with tc.tile_pool(name="sp4", bufs=1, space="PSUM") as sp4:
                ps4 = sp4.tile([H, 32, 16], F32)
                for widx in (0, 1):
                    for kx in range(3):
                        o = kx - 1
                        lhsT = wl4_sb[:, (widx * 3 + kx) * H : (widx * 3 + kx + 1) * H]
                        rhs = sl3[:, widx, :, 1 + o : 17 + o]
                        nc.tensor.matmul(
                            ps4[:, :, :],
                            lhsT,
                            rhs,
                            start=(widx == 0 and kx == 0),
                            stop=(widx == 1 and kx == 2),
                        )
                nc.vector.tensor_copy(out=sl4[:], in_=ps4[:])

            # border fix + projection for a pair (phase 4), pipelined per half
            def borders(p):
                for par in range(2):
                    bl = p * 4 + par * 2
                    br = bl + 1
                    po = par * H
                    if par == 0:
                        nc.vector.tensor_copy(
                            out=gfv_all[0:H, p, 0:4], in_=sl4[:, bl, 0:4]
                        )
                        nc.vector.tensor_copy(
                            out=gfv_all[0:H, p, 508:512], in_=sl4[:, br, 12:16]
                        )
                    else:
                        # partition-shifting writes go via DMA
                        nc.sync.dma_start(
                            out=gfv_all[po : po + H, p, 0:4], in_=sl4[:, bl, 0:4]
                        )
                        nc.sync.dma_start(
                            out=gfv_all[po : po + H, p, 508:512],
                            in_=sl4[:, br, 12:16],
                        )

            def phase4(p, tpp, lgp, otp, gtp, sgp):
                gfvT = gtp.tile([128, 4, 128], BF16, tag="gfvT", name=f"gfvT{p}")
                for c4 in range(4):
                    tps = []
                    for par in range(2):  # concurrent row-tiled pair T0/T8
                        po = par * H
                        tp = tpp.tile([128, H], BF16, tag=f"tp{par}",
                                      name=f"tp{p}_{c4}_{par}")
                        nc.tensor.transpose(
                            tp[:],
                            gfv_all[po : po + H, p, c4 * 128 : (c4 + 1) * 128],
                            id2_sb[po : po + H, :],
                        )
                        tps.append(tp)
                    for par in range(2):
                        nc.vector.tensor_copy(
                            out=gfvT[:, c4, par * H : (par + 1) * H],
                            in_=tps[par][:],
                        )
                lg = lgp.tile([K, 128], F32, tag="lg", name=f"lg{p}")
                for c4 in range(4):
                    nc.tensor.matmul(
                        lg[:],
                        pwt_sb[:, c4 * K : (c4 + 1) * K],
                        gfvT[:, c4, :],
                        start=(c4 == 0),
                        stop=(c4 == 3),
                    )
                sg = sgp.tile([K, 128], F32, tag="sg", name=f"sg{p}")
                nc.scalar.activation(sg[:], lg[:], SIG, bias=pb_sb[:], scale=1.0)
                ot = otp.tile([128, K], F32, tag="ot", name=f"ot{p}")
                nc.tensor.transpose(ot[:], sg[:], idf_sb[:])
                ots = sgp.tile([128, K], F32, tag="ots", name=f"ots{p}")
                nc.vector.tensor_copy(out=ots[:], in_=ot[:])
                nc.sync.dma_start(
                    out=out_d[2 * p : 2 * p + 2].rearrange("b h k -> (b h) k"),
                    in_=ots[:],
                )

            with (
                tc.tile_pool(name="tppA", bufs=1, space="PSUM") as tppA,
                tc.tile_pool(name="lgpA", bufs=1, space="PSUM") as lgpA,
                tc.tile_pool(name="otpA", bufs=1, space="PSUM") as otpA,
                tc.tile_pool(name="gtpA", bufs=2) as gtpA,
                tc.tile_pool(name="sgpA", bufs=2) as sgpA,
            ):
                for p in range(4):
                    borders(p)
                    phase4(p, tppA, lgpA, otpA, gtpA, sgpA)

                stage_a_half(1, gap)

        # tail pairs get a deeper pipeline once the stage-A banks are free
        with (
            tc.tile_pool(name="tppB", bufs=2, space="PSUM") as tppB,
            tc.tile_pool(name="lgpB", bufs=2, space="PSUM") as lgpB,
            tc.tile_pool(name="otpB", bufs=2, space="PSUM") as otpB,
            tc.tile_pool(name="gtpB", bufs=2) as gtpB,
            tc.tile_pool(name="sgpB", bufs=2) as sgpB,
        ):
            for p in range(4, NPAIR):
                borders(p)
                phase4(p, tppB, lgpB, otpB, gtpB, sgpB)

    nc.compile()
    _PROG_CACHE["nc"] = nc
    return nc


def _input_maps(x, conv_w, proj_w, proj_b):
    wa_sb, wa4_sb, wl_sb, wl4_sb, pwt_sb, pb, id2, idf = _prep_weights(
        conv_w, proj_w, proj_b
    )
    per_core = {
        "wa": wa_sb, "wa4": wa4_sb, "wl": wl_sb, "wl4": wl4_sb,
        "pwt": pwt_sb, "pb": pb, "id2": id2, "idf": idf,
    }
    xb = np.asarray(x[:, :NS_USED]).astype(ml_dtypes.bfloat16)
    in_maps = []
    for c in range(NCORES):
        shard = np.ascontiguousarray(xb[c * BPC : (c + 1) * BPC])
        in_maps.append(dict(per_core, x9=shard))
    return in_maps, per_core


# ----------------------------------------------------------------------------
# entry point
# ----------------------------------------------------------------------------
def kernel(x, conv_w, proj_w, proj_b, nslice=13, **_ignored):
    global LAST_EXEC_NS
    x = np.asarray(x, dtype=np.float32)
    nc = _build_program()
    in_maps, _ = _input_maps(x, conv_w, proj_w, proj_b)
    res = run_bass_kernel_spmd(
        nc, in_maps, list(range(NCORES)), trace=TRACE, tmpdir=TRACE_DIR
    )
    LAST_EXEC_NS = res.exec_time_ns
    out = np.concatenate([np.asarray(r["out"]) for r in res.results], axis=0)
    return out.astype(np.float32)


def bench(np_inputs, iters=32):
    """Estimate per-execution HW time by timing repeated async dispatches of
    the compiled NEFF with device-resident inputs (no output donation)."""
    import jax
    from jax.sharding import Mesh, PartitionSpec, NamedSharding
    from concourse import bass2jax as b2j
    from concourse import mybir as _mb

    b2j.install_neuronx_cc_hook()
    x = np.asarray(np_inputs["x"], dtype=np.float32)
    nc = _build_program()
    _, per_core = _input_maps(x, np_inputs["conv_w"], np_inputs["proj_w"],
                              np_inputs["proj_b"])

    in_names, out_names, out_avals, zero_outs = [], [], [], []
    for alloc in nc.m.functions[0].allocations:
        if not isinstance(alloc, _mb.MemoryLocationSet):
            continue
        name = alloc.memorylocations[0].name
        if alloc.kind == "ExternalInput":
            in_names.append(name)
        elif alloc.kind == "ExternalOutput":
            shape = tuple(alloc.tensor_shape)
            dtype = _mb.dt.np(alloc.dtype)
            out_names.append(name)
            out_avals.append(jax.core.ShapedArray(shape, dtype))
            zero_outs.append(np.zeros(shape, dtype))
    n_params = len(in_names)
    all_names = in_names + out_names

    def _body(*args):
        outs = b2j._bass_exec_p.bind(
            *args,
            out_avals=tuple(out_avals),
            in_names=tuple(all_names),
            out_names=tuple(out_names),
            lowering_input_output_aliases=(),
            sim_require_finite=True,
            sim_require_nnan=True,
            nc=nc,
        )
        return tuple(outs)

    devices = jax.devices()[:NCORES]
    mesh = Mesh(np.asarray(devices), ("core",))
    spec = PartitionSpec("core")
    from jax.experimental.shard_map import shard_map

    fn = jax.jit(
        shard_map(
            _body,
            mesh=mesh,
            in_specs=(spec,) * (n_params + len(out_names)),
            out_specs=(spec,) * len(out_names),
            check_rep=False,
        ),
        keep_unused=True,
    )

    xb = np.asarray(x[:, :NS_USED]).astype(ml_dtypes.bfloat16)
    concat_in = []
    for name in in_names:
        if name == "x9":
            arrs = [
                np.ascontiguousarray(xb[c * BPC : (c + 1) * BPC])
                for c in range(NCORES)
            ]
            concat_in.append(np.concatenate(arrs, axis=0))
        else:
            a = per_core[name]
            concat_in.append(np.concatenate([a] * NCORES, axis=0))
    concat_zeros = [
        np.zeros((NCORES * z.shape[0], *z.shape[1:]), z.dtype) for z in zero_outs
    ]
    sh = NamedSharding(mesh, spec)
    dev_args = [jax.device_put(a, sh) for a in concat_in + concat_zeros]

    r = fn(*dev_args)
    jax.block_until_ready(r)
    t0 = time.perf_counter()
    rs = None
    for _ in range(iters):
        rs = fn(*dev_args)
    jax.block_until_ready(rs)
    t1 = time.perf_counter()
    return (t1 - t0) / iters * 1e9


if __name__ == "__main__":
    xs = np.random.randn(NB, 13, H, W).astype(np.float32)
    cw = (np.random.randn(1, 3, 3, 3) * 0.1).astype(np.float32)
    pw = (np.random.randn(K, W) / np.sqrt(W)).astype(np.float32)
    pbb = (np.random.randn(K) * 0.01).astype(np.float32)
    o = kernel(xs, cw, pw, pbb, 13)
    print(o.shape, o.dtype)


# revision 16
# speedup vs baseline: 1.0840x; 1.0840x over previous
"""Trainium2 Bass kernel for the segment_reduce conv-pyramid problem.

Math: the reference applies 4 levels of a shared Conv2d(3->1, 3x3, SAME) over
sliding windows of the slice axis (13 -> 10 -> 7 -> 4 -> 1), then projects
W (512) down to K=10 and applies sigmoid.  Only x[:, 0:9] influences the
output.  The 4-level pyramid composes into

    gfv[b,h,w] = sum_{s=0..8} sum_{d=0..8} (M[s,d].T @ x[b,s])[h, w+d-4]

where M[s,d] are 64x64 matrices (exact in H, including H borders, computed
by composing the per-level banded matrices).  This is exact for interior
columns w in [4, 507]; the 8 border columns are computed exactly with a
direct 4-level recursion on width-16 strips (batched over all samples).

Performance structure (v2):
  - 2 samples share each PSUM bank (partitions 0:64 / 64:128); their
    stage-A matmuls are issued as concurrent column-tiled pairs
    (tile_position (0,0)/(0,64) for the 128-contraction groups,
    (0,0)/(64,64) for the single-slice group) -> ~2x PE throughput.
  - weight-block-outer loop (45 blocks x 8 in-flight samples) maximizes
    stationary-weight locality.
  - strips framed by memset (no zero-DMA); borders patched into the
    pair-stacked gfv tile; projection transposes run as concurrent
    row-tiled pairs (T0/T8); output DMA'd per-pair straight from the
    sigmoid tile.

Sharding: pure data parallel over the sample axis: 16 samples per core.
"""

import sys
import time

sys.path.insert(0, "/opt/trn_rl_repo")

import numpy as np  # noqa: E402
import ml_dtypes  # noqa: E402
from contextlib import ExitStack  # noqa: E402

import concourse.bass as bass  # noqa: E402
import concourse.bacc as bacc  # noqa: E402
import concourse.tile as tile  # noqa: E402
from concourse import mybir  # noqa: E402
from concourse.bass_utils import run_bass_kernel_spmd  # noqa: E402

NCORES = 8
NB, H, W, K = 128, 64, 512, 10
NS_USED = 9          # slices 9..12 never reach the output
BPC = NB // NCORES   # 16 samples per core
NPAIR = BPC // 2     # 8 sample-pairs per core

F32 = mybir.dt.float32
BF16 = mybir.dt.bfloat16
SIG = mybir.ActivationFunctionType.Sigmoid

TRACE = False
TRACE_DIR = None
LAST_EXEC_NS = None
_PROG_CACHE = {}

D_ORDER = (4, 0, 1, 2, 3, 5, 6, 7, 8)  # zero-shift first: full-width PSUM init


# ----------------------------------------------------------------------------
# host-side weight preparation (tiny, O(levels * 81 * 64^3))
# ----------------------------------------------------------------------------
def _banded(Kw, c, kx):
    """B[c,kx]: 64x64 lhsT-oriented [h_in, h_out] banded matrix."""
    B = np.zeros((H, H))
    for ho in range(H):
        for hi in range(max(0, ho - 1), min(H, ho + 2)):
            B[hi, ho] = Kw[c, hi - ho + 1, kx]
    return B


def _prep_weights(conv_w, proj_w, proj_b):
    Kw = np.asarray(conv_w, dtype=np.float64)[0]  # (3,3,3) [c, ky, kx]
    B = np.zeros((3, 3, H, H))
    for c in range(3):
        for kx in range(3):
            B[c, kx] = _banded(Kw, c, kx)

    # compose 4 levels: state (slice offset s, total x-shift d) -> matrix
    cur = {(0, 0): np.eye(H)}
    for _ in range(4):
        nxt = {}
        for (s, d), Mat in cur.items():
            for c in range(3):
                for kx in range(3):
                    key = (s + c, d + kx)
                    v = Mat @ B[c, kx]
                    if key in nxt:
                        nxt[key] = nxt[key] + v
                    else:
                        nxt[key] = v
        cur = nxt
    M = np.zeros((9, 9, H, H))
    for (s, d), Mat in cur.items():
        M[s, d] = Mat

    # stage-A weights for slice-pair groups g<4: wa[(g*9+d)] = [M[2g,d];
    # M[2g+1,d]] stacked rows, packed into SBUF layout [128, 36*64]
    wa = np.zeros((4, 9, 128, H), np.float32)
    for g in range(4):
        for d in range(9):
            wa[g, d, :H] = M[2 * g, d]
            wa[g, d, H:] = M[2 * g + 1, d]
    wa_sb = np.ascontiguousarray(
        wa.reshape(36, 128, H).transpose(1, 0, 2).reshape(128, 36 * H)
    ).astype(ml_dtypes.bfloat16)

    # stage-A weights for the single-slice group g=4, duplicated in both
    # partition halves so tile T10 (rows 64:128) can read them: [128, 9*64]
    wa4 = np.zeros((9, 128, H), np.float32)
    for d in range(9):
        wa4[d, 0:H] = M[8, d]
        wa4[d, H:] = M[8, d]
    wa4_sb = np.ascontiguousarray(
        wa4.transpose(1, 0, 2).reshape(128, 9 * H)
    ).astype(ml_dtypes.bfloat16)

    # strip level weights (pair-structured), SBUF layout [128, 6*128]
    wl = np.zeros((2, 3, 128, 128), np.float32)
    for kx in range(3):
        wl[0, kx, 0:H, 0:H] = B[0, kx]
        wl[0, kx, H:, 0:H] = B[1, kx]
        wl[0, kx, H:, H:] = B[0, kx]
        wl[1, kx, 0:H, 0:H] = B[2, kx]
        wl[1, kx, 0:H, H:] = B[1, kx]
        wl[1, kx, H:, H:] = B[2, kx]
    wl_sb = np.ascontiguousarray(
        wl.reshape(6, 128, 128).transpose(1, 0, 2).reshape(128, 6 * 128)
    ).astype(ml_dtypes.bfloat16)

    # level-4 weights (single output slice): [ [B0;B1], [B2;0] ] -> [128, 6*64]
    wl4 = np.zeros((2, 3, 128, H), np.float32)
    for kx in range(3):
        wl4[0, kx, 0:H] = B[0, kx]
        wl4[0, kx, H:] = B[1, kx]
        wl4[1, kx, 0:H] = B[2, kx]
    wl4_sb = np.ascontiguousarray(
        wl4.reshape(6, 128, H).transpose(1, 0, 2).reshape(128, 6 * H)
    ).astype(ml_dtypes.bfloat16)

    # projection weights, transposed per 128-chunk: [128, 4*K] bf16
    pw = np.asarray(proj_w, np.float64)  # [K, 512]
    pwt = np.zeros((128, 4 * K), np.float64)
    for c4 in range(4):
        pwt[:, c4 * K : (c4 + 1) * K] = pw[:, c4 * 128 : (c4 + 1) * 128].T
    pwt_sb = pwt.astype(ml_dtypes.bfloat16)

    pb = np.asarray(proj_b, np.float32).reshape(K, 1)

    # identity duplicated in both partition halves for row-tiled transposes
    id2 = np.zeros((128, H), np.float32)
    id2[0:H] = np.eye(H)
    id2[H:] = np.eye(H)
    id2_sb = id2.astype(ml_dtypes.bfloat16)
    idf = np.eye(K, dtype=np.float32)
    return wa_sb, wa4_sb, wl_sb, wl4_sb, pwt_sb, pb, id2_sb, idf


# ----------------------------------------------------------------------------
# device program (SPMD, identical on all 8 cores)
# ----------------------------------------------------------------------------
def _build_program():
    if "nc" in _PROG_CACHE:
        return _PROG_CACHE["nc"]

    nc = bacc.Bacc("TRN2", target_bir_lowering=False, debug=False)
    x9 = nc.dram_tensor("x9", [BPC, NS_USED, H, W], BF16, kind="ExternalInput").ap()
    wa_d = nc.dram_tensor("wa", [128, 36 * H], BF16, kind="ExternalInput").ap()
    wa4_d = nc.dram_tensor("wa4", [128, 9 * H], BF16, kind="ExternalInput").ap()
    wl_d = nc.dram_tensor("wl", [128, 6 * 128], BF16, kind="ExternalInput").ap()
    wl4_d = nc.dram_tensor("wl4", [128, 6 * H], BF16, kind="ExternalInput").ap()
    pwt_d = nc.dram_tensor("pwt", [128, 4 * K], BF16, kind="ExternalInput").ap()
    pb_d = nc.dram_tensor("pb", [K, 1], F32, kind="ExternalInput").ap()
    id2_d = nc.dram_tensor("id2", [128, H], BF16, kind="ExternalInput").ap()
    idf_d = nc.dram_tensor("idf", [K, K], F32, kind="ExternalInput").ap()
    out_d = nc.dram_tensor("out", [BPC, H, K], F32, kind="ExternalOutput").ap()

    with tile.TileContext(nc) as tc, ExitStack() as ctx:
        wp = ctx.enter_context(tc.tile_pool(name="wp", bufs=1))
        sp = ctx.enter_context(tc.tile_pool(name="sp", bufs=1))
        xp = ctx.enter_context(tc.tile_pool(name="xp", bufs=1))
        x8p = ctx.enter_context(tc.tile_pool(name="x8p", bufs=1))

        wa_sb = wp.tile([128, 36 * H], BF16)
        wa4_sb = wp.tile([128, 9 * H], BF16)
        wl_sb = wp.tile([128, 6 * 128], BF16)
        wl4_sb = wp.tile([128, 6 * H], BF16)
        pwt_sb = wp.tile([128, 4 * K], BF16)
        pb_sb = wp.tile([K, 1], F32)
        id2_sb = wp.tile([128, H], BF16)
        idf_sb = wp.tile([K, K], F32)
        for dst, src in (
            (wa_sb, wa_d), (wa4_sb, wa4_d), (wl_sb, wl_d), (wl4_sb, wl4_d),
            (pwt_sb, pwt_d), (pb_sb, pb_d), (id2_sb, id2_d), (idf_sb, idf_d),
        ):
            nc.sync.dma_start(out=dst[:], in_=src)

        # persistent strip tiles: [128, ntiles, 32 blocks, 18] where the 18-dim
        # is 16 data cols framed by one zero col each side (exact zero-pad for
        # the W-direction conv shifts).  block = pair*4 + parity*2 + side.
        sx = sp.tile([128, 5, 32, 18], BF16)
        sl1 = sp.tile([128, 4, 32, 18], BF16)
        sl2 = sp.tile([128, 3, 32, 18], BF16)
        sl3 = sp.tile([128, 2, 32, 18], BF16)
        sl4 = sp.tile([H, 32, 16], BF16)
        # pair-stacked gfv: partitions 0:64 even sample, 64:128 odd sample
        gfv_all = sp.tile([128, NPAIR, W], BF16)
        for t in (sx, sl1, sl2, sl3):
            nc.vector.memset(t[:], 0.0)

        # ------------------- loads + strip extraction -----------------------
        # few, large DMAs: ~1us SWDGE issue cost per dma_start dominates many
        # small transfers.  half 0's xm arrives in two g-chunks so stage A can
        # start early; half 1 arrives as whole pairs.  slice-8 tiles go on the
        # sync queue so both issue engines work in parallel.
        xm_t, x8_t = [], []
        for p in range(NPAIR):
            xm = xp.tile([128, 2, 4, W], BF16, tag=f"xm{p}")
            x8st = x8p.tile([128, W], BF16, tag=f"x8{p}")
            xm_t.append(xm)
            x8_t.append(x8st)

        def xm_dma(p, par, glo, ghi):
            nc.gpsimd.dma_start(
                out=xm_t[p][:, par, glo:ghi, :],
                in_=x9[2 * p + par, 2 * glo : 2 * ghi].rearrange(
                    "(g s) h w -> (s h) g w", s=2
                ),
            )

        for p in range(4):
            for par in range(2):
                xm_dma(p, par, 0, 2)
        for p in range(4):
            for par in range(2):
                xm_dma(p, par, 2, 4)
        for p in range(4, NPAIR):
            for par in range(2):
                xm_dma(p, par, 0, 4)
        for p in range(NPAIR):
            for par in range(2):
                nc.sync.dma_start(
                    out=x8_t[p][par * H : (par + 1) * H, :],
                    in_=x9[2 * p + par, 8],
                )

        for p in range(NPAIR):
            for par in range(2):
                for side in range(2):
                    blk = p * 4 + par * 2 + side
                    woff = 0 if side == 0 else W - 16
                    nc.vector.tensor_copy(
                        out=sx[:, 0:4, blk, 1:17],
                        in_=xm_t[p][:, par, 0:4, woff : woff + 16],
                    )
                    if par == 0:
                        nc.vector.tensor_copy(
                            out=sx[0:H, 4, blk, 1:17],
                            in_=x8_t[p][0:H, woff : woff + 16],
                        )
                    else:  # partition-shifting read goes via DMA
                        nc.sync.dma_start(
                            out=sx[0:H, 4, blk, 1:17],
                            in_=x8_t[p][H:, woff : woff + 16],
                        )

        # ------------------- stage A + strips + projection ------------------
        def stage_a_half(h, gap):
            """45 weight blocks x 8 samples; 2 samples per PSUM bank with
            concurrent column-tiled matmul pairs."""
            banks = []
            for p4 in range(4):
                gbank = gap.tile([128, W], F32, tag=f"g{p4}", name=f"gb{h}_{p4}")
                banks.append(gbank)
            for g in range(5):
                for d in D_ORDER:
                    o = d - 4
                    ol, oh = max(0, -o), W - max(0, o)
                    for p4 in range(4):
                        hp = 4 * h + p4
                        for par in range(2):
                            po = par * H
                            if g < 4:
                                lhsT = wa_sb[:, (g * 9 + d) * H : (g * 9 + d + 1) * H]
                                rhs = xm_t[hp][:, par, g, ol + o : oh + o]
                                tpos = (0, po)
                            else:
                                lhsT = wa4_sb[po : po + H, d * H : (d + 1) * H]
                                rhs = x8_t[hp][po : po + H, ol + o : oh + o]
                                tpos = (po, po)
                            nc.tensor.matmul(
                                banks[p4][po : po + H, ol:oh],
                                lhsT,
                                rhs,
                                start=(g == 0 and d == 4),
                                stop=(g == 4 and d == 8),
                                tile_position=tpos,
                            )
            for p4 in range(4):
                hp = 4 * h + p4
                for par in range(2):
                    po = par * H
                    nc.vector.tensor_copy(
                        out=gfv_all[po : po + H, hp, :],
                        in_=banks[p4][po : po + H, :],
                    )

        def strip_level(dst, src, src_ntile, n_out):
            """One pyramid level on the strips. dst gets ceil(n_out/2) tiles."""
            with tc.tile_pool(name=f"spp{n_out}", bufs=4, space="PSUM") as spp:
                for t in range((n_out + 1) // 2):
                    single = (2 * t + 1) >= n_out
                    cols = H if single else 128
                    ps = spp.tile([128, 32, 16], F32)
                    for widx in (0, 1):  # W1 @ src[t], W2 @ src[t+1]
                        it = t + widx
                        if it >= src_ntile:
                            continue
                        for kx in range(3):
                            o = kx - 1
                            rhs = src[:, it, :, 1 + o : 17 + o]
                            for ch in range(cols // H):
                                co = ch * H
                                lhsT = wl_sb[:, (widx * 3 + kx) * 128 + co :][:, 0:H]
                                nc.tensor.matmul(
                                    ps[co : co + H, :, :],
                                    lhsT,
                                    rhs,
                                    start=(widx == 0 and kx == 0),
                                    stop=(widx == 1 and kx == 2),
                                    tile_position=(0, co),
                                )
                    nc.vector.tensor_copy(
                        out=dst[0:cols, t, :, 1:17], in_=ps[0:cols, :, :]
                    )

        with tc.tile_pool(name="gap", bufs=1, space="PSUM") as gap:
            stage_a_half(0, gap)

            # exact border strips (levels 1..4), PSUM banks 4..7
            strip_level(sl1, sx, 5, 7)
            strip_level(sl2, sl1, 4, 5)
            strip_level(sl3, sl2, 3, 3)
            with tc.tile_pool(name="sp4", bufs=1, space="PSUM") as sp4:
                ps4 = sp4.tile([H, 32, 16], F32)
                for widx in (0, 1):
                    for kx in range(3):
                        o = kx - 1
                        lhsT = wl4_sb[:, (widx * 3 + kx) * H : (widx * 3 + kx + 1) * H]
                        rhs = sl3[:, widx, :, 1 + o : 17 + o]
                        nc.tensor.matmul(
                            ps4[:, :, :],
                            lhsT,
                            rhs,
                            start=(widx == 0 and kx == 0),
                            stop=(widx == 1 and kx == 2),
                        )
                nc.vector.tensor_copy(out=sl4[:], in_=ps4[:])

            # border fix + projection for a pair (phase 4), pipelined per half
            def borders(p):
                for par in range(2):
                    bl = p * 4 + par * 2
                    br = bl + 1
                    po = par * H
                    if par == 0:
                        nc.vector.tensor_copy(
                            out=gfv_all[0:H, p, 0:4], in_=sl4[:, bl, 0:4]
                        )
                        nc.vector.tensor_copy(
                            out=gfv_all[0:H, p, 508:512], in_=sl4[:, br, 12:16]
                        )
                    else:
                        # partition-shifting writes go via DMA
                        nc.sync.dma_start(
                            out=gfv_all[po : po + H, p, 0:4], in_=sl4[:, bl, 0:4]
                        )
                        nc.sync.dma_start(
                            out=gfv_all[po : po + H, p, 508:512],
                            in_=sl4[:, br, 12:16],
                        )

            def phase4(p, tpp, lgp, otp, gtp, sgp):
                gfvT = gtp.tile([128, 4, 128], BF16, tag="gfvT", name=f"gfvT{p}")
                for c4 in range(4):
                    tps = []
                    for par in range(2):  # concurrent row-tiled pair T0/T8
                        po = par * H
                        tp = tpp.tile([128, H], BF16, tag=f"tp{par}",
                                      name=f"tp{p}_{c4}_{par}")
                        nc.tensor.transpose(
                            tp[:],
                            gfv_all[po : po + H, p, c4 * 128 : (c4 + 1) * 128],
                            id2_sb[po : po + H, :],
                        )
                        tps.append(tp)
                    for par in range(2):
                        nc.vector.tensor_copy(
                            out=gfvT[:, c4, par * H : (par + 1) * H],
                            in_=tps[par][:],
                        )
                lg = lgp.tile([K, 128], F32, tag="lg", name=f"lg{p}")
                for c4 in range(4):
                    nc.tensor.matmul(
                        lg[:],
                        pwt_sb[:, c4 * K : (c4 + 1) * K],
                        gfvT[:, c4, :],
                        start=(c4 == 0),
                        stop=(c4 == 3),
                    )
                sg = sgp.tile([K, 128], F32, tag="sg", name=f"sg{p}")
                nc.scalar.activation(sg[:], lg[:], SIG, bias=pb_sb[:], scale=1.0)
                ot = otp.tile([128, K], F32, tag="ot", name=f"ot{p}")
                nc.tensor.transpose(ot[:], sg[:], idf_sb[:])
                ots = sgp.tile([128, K], F32, tag="ots", name=f"ots{p}")
                nc.vector.tensor_copy(out=ots[:], in_=ot[:])
                nc.sync.dma_start(
                    out=out_d[2 * p : 2 * p + 2].rearrange("b h k -> (b h) k"),
                    in_=ots[:],
                )

            with (
                tc.tile_pool(name="tppA", bufs=1, space="PSUM") as tppA,
                tc.tile_pool(name="lgpA", bufs=1, space="PSUM") as lgpA,
                tc.tile_pool(name="otpA", bufs=1, space="PSUM") as otpA,
                tc.tile_pool(name="gtpA", bufs=2) as gtpA,
                tc.tile_pool(name="sgpA", bufs=2) as sgpA,
            ):
                for p in range(4):
                    borders(p)
                    phase4(p, tppA, lgpA, otpA, gtpA, sgpA)

                stage_a_half(1, gap)

        # tail pairs get a deeper pipeline once the stage-A banks are free
        with (
            tc.tile_pool(name="tppB", bufs=2, space="PSUM") as tppB,
            tc.tile_pool(name="lgpB", bufs=2, space="PSUM") as lgpB,
            tc.tile_pool(name="otpB", bufs=2, space="PSUM") as otpB,
            tc.tile_pool(name="gtpB", bufs=2) as gtpB,
            tc.tile_pool(name="sgpB", bufs=2) as sgpB,
        ):
            for p in range(4, NPAIR):
                borders(p)
                phase4(p, tppB, lgpB, otpB, gtpB, sgpB)

    nc.compile()
    _PROG_CACHE["nc"] = nc
    return nc


def _input_maps(x, conv_w, proj_w, proj_b):
    wa_sb, wa4_sb, wl_sb, wl4_sb, pwt_sb, pb, id2, idf = _prep_weights(
        conv_w, proj_w, proj_b
    )
    per_core = {
        "wa": wa_sb, "wa4": wa4_sb, "wl": wl_sb, "wl4": wl4_sb,
        "pwt": pwt_sb, "pb": pb, "id2": id2, "idf": idf,
    }
    xb = np.asarray(x[:, :NS_USED]).astype(ml_dtypes.bfloat16)
    in_maps = []
    for c in range(NCORES):
        shard = np.ascontiguousarray(xb[c * BPC : (c + 1) * BPC])
        in_maps.append(dict(per_core, x9=shard))
    return in_maps, per_core


# ----------------------------------------------------------------------------
# entry point
# ----------------------------------------------------------------------------
def kernel(x, conv_w, proj_w, proj_b, nslice=13, **_ignored):
    global LAST_EXEC_NS
    x = np.asarray(x, dtype=np.float32)
    nc = _build_program()
    in_maps, _ = _input_maps(x, conv_w, proj_w, proj_b)
    res = run_bass_kernel_spmd(
        nc, in_maps, list(range(NCORES)), trace=TRACE, tmpdir=TRACE_DIR
    )
    LAST_EXEC_NS = res.exec_time_ns
    out = np.concatenate([np.asarray(r["out"]) for r in res.results], axis=0)
    return out.astype(np.float32)


def bench(np_inputs, iters=32):
    """Estimate per-execution HW time by timing repeated async dispatches of
    the compiled NEFF with device-resident inputs (no output donation)."""
    import jax
    from jax.sharding import Mesh, PartitionSpec, NamedSharding
    from concourse import bass2jax as b2j
    from concourse import mybir as _mb

    b2j.install_neuronx_cc_hook()
    x = np.asarray(np_inputs["x"], dtype=np.float32)
    nc = _build_program()
    _, per_core = _input_maps(x, np_inputs["conv_w"], np_inputs["proj_w"],
                              np_inputs["proj_b"])

    in_names, out_names, out_avals, zero_outs = [], [], [], []
    for alloc in nc.m.functions[0].allocations:
        if not isinstance(alloc, _mb.MemoryLocationSet):
            continue
        name = alloc.memorylocations[0].name
        if alloc.kind == "ExternalInput":
            in_names.append(name)
        elif alloc.kind == "ExternalOutput":
            shape = tuple(alloc.tensor_shape)
            dtype = _mb.dt.np(alloc.dtype)
            out_names.append(name)
            out_avals.append(jax.core.ShapedArray(shape, dtype))
            zero_outs.append(np.zeros(shape, dtype))
    n_params = len(in_names)
    all_names = in_names + out_names

    def _body(*args):
        outs = b2j._bass_exec_p.bind(
            *args,
            out_avals=tuple(out_avals),
            in_names=tuple(all_names),
            out_names=tuple(out_names),
            lowering_input_output_aliases=(),
            sim_require_finite=True,
            sim_require_nnan=True,
            nc=nc,
        )
        return tuple(outs)

    devices = jax.devices()[:NCORES]
    mesh = Mesh(np.asarray(devices), ("core",))
    spec = PartitionSpec("core")
    from jax.experimental.shard_map import shard_map

    fn = jax.jit(
        shard_map(
            _body,
            mesh=mesh,
            in_specs=(spec,) * (n_params + len(out_names)),
            out_specs=(spec,) * len(out_names),
            check_rep=False,
        ),
        keep_unused=True,
    )

    xb = np.asarray(x[:, :NS_USED]).astype(ml_dtypes.bfloat16)
    concat_in = []
    for name in in_names:
        if name == "x9":
            arrs = [
                np.ascontiguousarray(xb[c * BPC : (c + 1) * BPC])
                for c in range(NCORES)
            ]
            concat_in.append(np.concatenate(arrs, axis=0))
        else:
            a = per_core[name]
            concat_in.append(np.concatenate([a] * NCORES, axis=0))
    concat_zeros = [
        np.zeros((NCORES * z.shape[0], *z.shape[1:]), z.dtype) for z in zero_outs
    ]
    sh = NamedSharding(mesh, spec)
    dev_args = [jax.device_put(a, sh) for a in concat_in + concat_zeros]

    r = fn(*dev_args)
    jax.block_until_ready(r)
    t0 = time.perf_counter()
    rs = None
    for _ in range(iters):
        rs = fn(*dev_args)
    jax.block_until_ready(rs)
    t1 = time.perf_counter()
    return (t1 - t0) / iters * 1e9


if __name__ == "__main__":
    xs = np.random.randn(NB, 13, H, W).astype(np.float32)
    cw = (np.random.randn(1, 3, 3, 3) * 0.1).astype(np.float32)
    pw = (np.random.randn(K, W) / np.sqrt(W)).astype(np.float32)
    pbb = (np.random.randn(K) * 0.01).astype(np.float32)
    o = kernel(xs, cw, pw, pbb, 13)
    print(o.shape, o.dtype)
